# revision 25
# baseline (speedup 1.0000x reference)
"""Trainium2 Bass kernel for nn_ConnectLoss.

loss = sum(relu(|x[:,j] - x[:,j-1]| - 1) * mask[:,j]) over j in [1, L).

Pure data-parallel over 8 NeuronCores: rows sharded 8192/core. Inputs
are downcast to bf16 on host (verified rel err ~5e-5 vs the 2e-2
gate), halving HBM traffic; per-core DRAM layout is partition-major
(partition p owns 64 adjacent rows) so every DMA moves one contiguous
8KB run per partition (128 descriptors / MB instead of 512).

Per 8-row megatile [128, 8, 512]:
  DVE tensor_tensor      dd[:, :, 0:511] = x[:,1:] - x[:,:-1]   (2x)
  ACT activation(Abs)    dd = |dd|            (in-place)
  DVE tensor_scalar      dd = max(dd - 1, 0)  (in-place, (add,max), 4x)
  DVE tensor_tensor      prod = dd * mshift   (2x)
  PE  matmul x8          ones[128,1].T @ prod[:, r, :] accumulating
                         into one PSUM bank [1, 512] f32
The masked sum runs on the otherwise-idle PE (the 1x-rate DVE
scalar_tensor_tensor accumulate was the bottleneck); one final ACT
Copy reduces PSUM -> acc[1, 1]. mshift is the mask DMA'd with a
one-element shift so DVE passes run full-width and 4B-aligned (16-bit
packed modes); dd col 511 is zeroed once per buffer so prod col 511
is 0 and the junk mshift tail column contributes nothing. Host sums
the 8 per-core scalars in f64.
"""
import sys

sys.path.insert(0, "/opt/trn_rl_repo")
import numpy as np
import ml_dtypes

N_CORES = 8
M_ROWS = 65536
LENGTH = 512
ROWS_PER_CORE = M_ROWS // N_CORES
P = 128
RPP = ROWS_PER_CORE // P  # rows per partition (64)
BLOCKS = 4  # rows-per-partition fused per megatile
N_MEGA = RPP // BLOCKS
MASK_PAD = 64  # flat mask is read at +1 element offset

_nc_cache = None


def _build_nc():
    import concourse.tile as tile
    import concourse.mybir as mybir
    from concourse import bacc

    nc = bacc.Bacc(None)
    f32 = mybir.dt.float32
    bf16 = mybir.dt.bfloat16
    n_elem = ROWS_PER_CORE * LENGTH
    x = nc.declare_dram_parameter("x", [n_elem], bf16, isOutput=False)
    msk = nc.declare_dram_parameter(
        "mask", [n_elem + MASK_PAD], bf16, isOutput=False
    )
    out = nc.declare_dram_parameter("out", [1, 1], f32, isOutput=True)

    xv = x[0:n_elem].rearrange("(p r c) -> p r c", p=P, r=RPP, c=LENGTH)
    mv = msk[1 : 1 + n_elem].rearrange("(p r c) -> p r c", p=P, r=RPP, c=LENGTH)

    L1 = LENGTH - 1
    with tile.TileContext(nc) as tc:
        with (
            tc.tile_pool(name="xin", bufs=6) as xpool,
            tc.tile_pool(name="min", bufs=6) as mpool,
            tc.tile_pool(name="work", bufs=1) as wpool,
            tc.tile_pool(name="prodp", bufs=4) as ppool,
            tc.tile_pool(name="acc", bufs=1) as apool,
            tc.psum_pool(name="ps", bufs=1) as pspool,
        ):
            acc = apool.tile([1, 1], f32, tag="acc")
            ones = apool.tile([P, 1], bf16, tag="ones")
            nc.vector.memset(ones[:], 1.0)
            ps = pspool.tile([1, LENGTH], f32, tag="ps")
            dds = [
                wpool.tile(
                    [P, BLOCKS, LENGTH], bf16, tag=f"dd{i}", name=f"dd{i}"
                )
                for i in range(4)
            ]
            for dd in dds:
                nc.vector.memset(dd[:, :, L1:LENGTH], 0.0)
            for t in range(N_MEGA):
                r0, r1 = t * BLOCKS, (t + 1) * BLOCKS
                xt = xpool.tile([P, BLOCKS, LENGTH], bf16, tag="xt")
                mt = mpool.tile([P, BLOCKS, LENGTH], bf16, tag="mt")
                nc.sync.dma_start(xt[:], xv[:, r0:r1, :])
                nc.scalar.dma_start(mt[:], mv[:, r0:r1, :])
                dd = dds[t % 4]
                nc.vector.tensor_tensor(
                    dd[:, :, 0:L1],
                    xt[:, :, 1:LENGTH],
                    xt[:, :, 0:L1],
                    mybir.AluOpType.subtract,
                )
                nc.scalar.activation(
                    dd[:],
                    dd[:],
                    mybir.ActivationFunctionType.Abs,
                )
                nc.vector.tensor_scalar(
                    dd[:],
                    dd[:],
                    -1.0,
                    0.0,
                    op0=mybir.AluOpType.add,
                    op1=mybir.AluOpType.max,
                )
                prod = ppool.tile([P, BLOCKS, LENGTH], bf16, tag="prod")
                nc.gpsimd.tensor_tensor(
                    prod[:],
                    dd[:],
                    mt[:],
                    mybir.AluOpType.mult,
                )
                for r in range(BLOCKS):
                    nc.tensor.matmul(
                        ps[:],
                        ones[:],
                        prod[:, r, :],
                        start=(t == 0 and r == 0),
                        stop=(t == N_MEGA - 1 and r == BLOCKS - 1),
                    )
            fin = apool.tile([1, LENGTH], f32, tag="fin")
            nc.scalar.activation(
                fin[:],
                ps[:],
                mybir.ActivationFunctionType.Copy,
                accum_out=acc[:],
            )
            nc.sync.dma_start(out[:], acc[:])
    nc.compile()
    return nc


def _get_nc():
    global _nc_cache
    if _nc_cache is None:
        _nc_cache = _build_nc()
    return _nc_cache


def _finish(outs) -> np.ndarray:
    o = np.stack(outs).astype(np.float64)  # [cores, 1, 1]
    return np.asarray(o.sum(), dtype=np.float32)


def run_spmd(x, mask, trace: bool = False):
    """Returns (loss ndarray, BassKernelResults)."""
    from concourse.bass_utils import run_bass_kernel_spmd

    bf16 = ml_dtypes.bfloat16
    x = np.asarray(x, dtype=np.float32).astype(bf16)
    mask = np.asarray(mask, dtype=np.float32).astype(bf16)
    assert x.shape == (M_ROWS, LENGTH) and mask.shape == (M_ROWS, LENGTH)

    pad = np.zeros([MASK_PAD], dtype=bf16)
    in_maps = []
    for i in range(N_CORES):
        r0, r1 = i * ROWS_PER_CORE, (i + 1) * ROWS_PER_CORE
        in_maps.append(
            {
                "x": np.ascontiguousarray(x[r0:r1]).reshape(-1),
                "mask": np.concatenate(
                    [np.ascontiguousarray(mask[r0:r1]).reshape(-1), pad]
                ),
            }
        )
    res = run_bass_kernel_spmd(
        _get_nc(), in_maps, list(range(N_CORES)), trace=trace
    )
    loss = _finish([r["out"] for r in res.results])
    return loss, res


def kernel(x, mask) -> np.ndarray:
    loss, _ = run_spmd(x, mask, trace=False)
    return loss


# revision 32
# speedup vs baseline: 1.8028x; 1.8028x over previous
"""Trainium2 Bass kernel for nn_ConnectLoss.

loss = sum(relu(|x[:,j] - x[:,j-1]| - 1) * mask[:,j]) over j in [1, L).

Pure data-parallel over 8 NeuronCores: rows sharded 8192/core. Inputs
are downcast to bf16 on host (verified rel err ~5e-5 vs the 2e-2
gate), halving HBM traffic; per-core DRAM layout is partition-major
(partition p owns 64 adjacent rows) so every DMA moves one contiguous
8KB run per partition (128 descriptors / MB instead of 512).

Per 4-row tile [128, 4, 512] (16 tiles/core):
  DVE tensor_tensor      dd[:, :, 0:511] = x[:,1:] - x[:,:-1]   (2x)
  ACT activation(Abs)    dd = |dd|            (in-place)
  DVE tensor_scalar      dd = max(dd - 1, 0)  (in-place, (add,max), 4x)
  DVE tensor_tensor      prod = dd * mshift   (2x)
  PE  matmul x4          ones[128,1].T @ prod[:, r, :] accumulating
                         into one PSUM bank [1, 512] f32
The masked sum runs on the otherwise-idle PE (the 1x-rate DVE
scalar_tensor_tensor accumulate was the bottleneck); one final ACT
Copy reduces PSUM -> acc[1, 1]. mshift is the mask DMA'd with a
one-element shift so DVE passes run full-width and 4B-aligned (16-bit
packed modes); dd col 511 is zeroed once per buffer so prod col 511
is 0 and the junk mshift tail column contributes nothing. Host sums
the 8 per-core scalars in f64.
"""
import sys

sys.path.insert(0, "/opt/trn_rl_repo")
import numpy as np
import ml_dtypes

N_CORES = 8
M_ROWS = 65536
LENGTH = 512
ROWS_PER_CORE = M_ROWS // N_CORES
P = 128
RPP = ROWS_PER_CORE // P  # rows per partition (64)
BLOCKS = 4  # rows-per-partition fused per megatile
N_MEGA = RPP // BLOCKS
MASK_PAD = 64  # flat mask is read at +1 element offset

_nc_cache = None


def _build_nc():
    import concourse.tile as tile
    import concourse.mybir as mybir
    from concourse import bacc

    nc = bacc.Bacc(None)
    f32 = mybir.dt.float32
    bf16 = mybir.dt.bfloat16
    n_elem = ROWS_PER_CORE * LENGTH
    x = nc.declare_dram_parameter("x", [n_elem], bf16, isOutput=False)
    msk = nc.declare_dram_parameter(
        "mask", [n_elem + MASK_PAD], bf16, isOutput=False
    )
    out = nc.declare_dram_parameter("out", [1, 1], f32, isOutput=True)

    xv = x[0:n_elem].rearrange("(p r c) -> p r c", p=P, r=RPP, c=LENGTH)
    mv = msk[1 : 1 + n_elem].rearrange("(p r c) -> p r c", p=P, r=RPP, c=LENGTH)

    L1 = LENGTH - 1
    with tile.TileContext(nc) as tc:
        with (
            tc.tile_pool(name="xin", bufs=8) as xpool,
            tc.tile_pool(name="min", bufs=6) as mpool,
            tc.tile_pool(name="work", bufs=1) as wpool,
            tc.tile_pool(name="prodp", bufs=6) as ppool,
            tc.tile_pool(name="acc", bufs=1) as apool,
            tc.psum_pool(name="ps", bufs=1) as pspool,
        ):
            acc = apool.tile([1, 1], f32, tag="acc")
            ones = apool.tile([P, 1], bf16, tag="ones")
            nc.vector.memset(ones[:], 1.0)
            ps = pspool.tile([1, LENGTH], f32, tag="ps")
            dds = [
                wpool.tile(
                    [P, BLOCKS, LENGTH], bf16, tag=f"dd{i}", name=f"dd{i}"
                )
                for i in range(6)
            ]
            for dd in dds:
                nc.vector.memset(dd[:, :, L1:LENGTH], 0.0)
            XLEAD = 2  # issue x DMAs 2 tiles ahead of mask DMAs
            xts = {}
            for t in range(XLEAD):
                r0, r1 = t * BLOCKS, (t + 1) * BLOCKS
                xts[t] = xpool.tile(
                    [P, BLOCKS, LENGTH], bf16, tag="xt", name=f"xt{t}"
                )
                nc.sync.dma_start(xts[t][:], xv[:, r0:r1, :])
            for t in range(N_MEGA):
                r0, r1 = t * BLOCKS, (t + 1) * BLOCKS
                ta = t + XLEAD
                if ta < N_MEGA:
                    xts[ta] = xpool.tile(
                        [P, BLOCKS, LENGTH], bf16, tag="xt", name=f"xt{ta}"
                    )
                    nc.sync.dma_start(
                        xts[ta][:], xv[:, ta * BLOCKS : (ta + 1) * BLOCKS, :]
                    )
                mt = mpool.tile([P, BLOCKS, LENGTH], bf16, tag="mt")
                nc.sync.dma_start(mt[:], mv[:, r0:r1, :])
                xt = xts.pop(t)
                dd = dds[t % 6]
                nc.vector.tensor_tensor(
                    dd[:, :, 0:L1],
                    xt[:, :, 1:LENGTH],
                    xt[:, :, 0:L1],
                    mybir.AluOpType.subtract,
                )
                nc.scalar.activation(
                    dd[:],
                    dd[:],
                    mybir.ActivationFunctionType.Abs,
                )
                nc.vector.tensor_scalar(
                    dd[:],
                    dd[:],
                    -1.0,
                    0.0,
                    op0=mybir.AluOpType.add,
                    op1=mybir.AluOpType.max,
                )
                prod = ppool.tile([P, BLOCKS, LENGTH], bf16, tag="prod")
                nc.gpsimd.tensor_tensor(
                    prod[:],
                    dd[:],
                    mt[:],
                    mybir.AluOpType.mult,
                )
                for r in range(BLOCKS):
                    nc.tensor.matmul(
                        ps[:],
                        ones[:],
                        prod[:, r, :],
                        start=(t == 0 and r == 0),
                        stop=(t == N_MEGA - 1 and r == BLOCKS - 1),
                    )
            fin = apool.tile([1, LENGTH], f32, tag="fin")
            nc.scalar.activation(
                fin[:],
                ps[:],
                mybir.ActivationFunctionType.Copy,
                accum_out=acc[:],
            )
            nc.sync.dma_start(out[:], acc[:])
    nc.compile()
    return nc


def _get_nc():
    global _nc_cache
    if _nc_cache is None:
        _nc_cache = _build_nc()
    return _nc_cache


def _finish(outs) -> np.ndarray:
    o = np.stack(outs).astype(np.float64)  # [cores, 1, 1]
    return np.asarray(o.sum(), dtype=np.float32)


def run_spmd(x, mask, trace: bool = False):
    """Returns (loss ndarray, BassKernelResults)."""
    from concourse.bass_utils import run_bass_kernel_spmd

    bf16 = ml_dtypes.bfloat16
    x = np.asarray(x, dtype=np.float32).astype(bf16)
    mask = np.asarray(mask, dtype=np.float32).astype(bf16)
    assert x.shape == (M_ROWS, LENGTH) and mask.shape == (M_ROWS, LENGTH)

    pad = np.zeros([MASK_PAD], dtype=bf16)
    in_maps = []
    for i in range(N_CORES):
        r0, r1 = i * ROWS_PER_CORE, (i + 1) * ROWS_PER_CORE
        in_maps.append(
            {
                "x": np.ascontiguousarray(x[r0:r1]).reshape(-1),
                "mask": np.concatenate(
                    [np.ascontiguousarray(mask[r0:r1]).reshape(-1), pad]
                ),
            }
        )
    res = run_bass_kernel_spmd(
        _get_nc(), in_maps, list(range(N_CORES)), trace=trace
    )
    loss = _finish([r["out"] for r in res.results])
    return loss, res


def kernel(x, mask) -> np.ndarray:
    loss, _ = run_spmd(x, mask, trace=False)
    return loss
